# revision 10
# baseline (speedup 1.0000x reference)
"""Trainium2 Bass kernel for:
    tanh( (x0*x1 + sin(x2)) * exp(-|x3|) + x4 / (x5*x5 + exp(x6)) - x7 )
over inputs (8388608, 8) f32, data-parallel over 8 NeuronCores.

v3 design (compute-bound after the fp16 shrink; DMA floor ~32 us):
  - Host marshals inputs to var-major fp16 (end-to-end error ~4e-4 vs
    the 2e-2 gate): xs[7, R] holds vars in order [x0,x1,x3,x6,x4,x5,x7]
    so the two exp operands (-|x3| and x6) are ADJACENT -> one fused
    2F-wide ACT exp instead of two F-wide ones. x2[R] fp16 separately.
  - Two ACT table-set phases TOTAL (not per batch): pass A computes
    sin(wrap(x2)) for the whole shard into a resident fp16 buffer
    (silu set), pass B does exp/square/tanh (exp_and_others set).
  - F=2048 tiles (4/core) halve per-instruction overheads.
  - In-place ops throughout (sign-or on the x3 slice, d/recip on the
    sq tile, the 5-op DVE chain on one w tile) keep SBUF under 180 KB
    per partition.
  - Engine balance per tile: ACT exp(2F)+square+tanh; Pool d,q;
    DVE sign-or, recip, 5 fp16 tensor_tensor at 2x mode.
"""

import numpy as np

import concourse.bass as bass
import concourse.bacc as bacc
import concourse.mybir as mybir
from concourse.tile import TileContext
from concourse.tile_rust import add_dep_helper
from concourse import bass_utils

N_ROWS = 8_388_608
N_VARS = 8
N_CORES = 8
ROWS_PER_CORE = N_ROWS // N_CORES  # 1_048_576
P = 128
F = 1024
TILE_ROWS = P * F                  # 131_072
N_TILES = ROWS_PER_CORE // TILE_ROWS  # 8

F32 = mybir.dt.float32
F16 = mybir.dt.float16
U16 = mybir.dt.uint16
AF = mybir.ActivationFunctionType
OP = mybir.AluOpType

# xs row order: x3 and x6 adjacent for the fused exp
XS_VARS = [0, 1, 3, 6, 4, 5, 7]
SLOT = {v: i for i, v in enumerate(XS_VARS)}


def build_bass(loop_iters: int = 1, ablate: str = "none") -> bass.Bass:
    import contextlib
    nc = bacc.Bacc("TRN2", debug=False, num_devices=N_CORES)
    xs = nc.dram_tensor("xs", [7, ROWS_PER_CORE], F16, kind="ExternalInput").ap()
    x2 = nc.dram_tensor("x2", [ROWS_PER_CORE], F16, kind="ExternalInput").ap()
    y = nc.dram_tensor("y", [ROWS_PER_CORE], F16, kind="ExternalOutput").ap()

    with TileContext(nc) as tc:
        with (
            tc.tile_pool(name="sin", bufs=1) as sin_pool,
            tc.tile_pool(name="pa", bufs=3) as pa_pool,
            tc.tile_pool(name="inp", bufs=5) as inp_pool,
            tc.tile_pool(name="tmp", bufs=3) as tmp_pool,
            (tc.For_i(0, loop_iters, 1) if loop_iters > 1
             else contextlib.nullcontext()),
        ):
            stile = sin_pool.tile([P, N_TILES * F], F16, name="stile")

            # ---- Pass A: sin(wrap(x2)) for the whole shard (silu set) ----
            # One up-front 2MB DMA so the sins are never starved behind the
            # big pass-B loads; wrap+sin in 2 wide chunks (fewer ops).
            x2all = sin_pool.tile([P, N_TILES * F], F16, name="x2all")
            nc.sync.dma_start(
                out=x2all.rearrange("p (t f) -> p t f", t=N_TILES),
                in_=x2.rearrange("(t p f) -> p t f", t=N_TILES, p=P))
            last_sin = None
            if ablate != "dma":
                CH = N_TILES * F // 2
                for c in range(2):
                    sl = slice(c * CH, (c + 1) * CH)
                    wr = pa_pool.tile([P, CH], F16, name=f"wr{c}", tag="wr")
                    nc.vector.add_range_wrap(
                        out=wr, in_=x2all[:, sl], shift=0.0,
                        bound=float(np.pi), period=float(2 * np.pi))
                    si = nc.scalar.activation(stile[:, sl], wr, AF.Sin)
                    last_sin = si.ins

            # ---- Pass B: everything else (exp_and_others set) ----
            for t in range(N_TILES):
                r0, r1 = t * TILE_ROWS, (t + 1) * TILE_ROWS
                xt = inp_pool.tile([P, 7 * F], F16, name=f"xt{t}", tag="xt")
                nc.sync.dma_start(
                    out=xt.rearrange("p (v f) -> p v f", v=7),
                    in_=xs[:, r0:r1].rearrange("v (p f) -> p v f", p=P))
                v = {k: xt[:, s * F:(s + 1) * F] for k, s in SLOT.items()}
                if ablate == "dma":
                    nc.sync.dma_start(
                        out=y[r0:r1].rearrange("(p f) -> p f", p=P),
                        in_=v[7])
                    continue

                # -|x3| in place via sign-bit OR (DVE, 16-bit)
                nc.vector.tensor_scalar(
                    out=v[3].bitcast(U16), in0=v[3].bitcast(U16),
                    scalar1=0x8000, scalar2=None, op0=OP.bitwise_or)

                # fused exp over the adjacent [x3'|x6] 2F slice
                e2 = tmp_pool.tile([P, 2 * F], F16, name=f"e2{t}", tag="e2")
                i1 = nc.scalar.activation(e2, xt[:, 2 * F:4 * F], AF.Exp)
                e, e6 = e2[:, 0:F], e2[:, F:2 * F]

                sq = tmp_pool.tile([P, F], F16, name=f"sq{t}", tag="sq")
                nc.vector.tensor_tensor(out=sq, in0=v[5], in1=v[5], op=OP.mult)

                d = tmp_pool.tile([P, F], F32, name=f"d{t}", tag="d")
                nc.gpsimd.tensor_tensor(out=d, in0=sq, in1=e6, op=OP.add)
                nc.vector.reciprocal_approx_fast(out=d, in_=d)
                q = tmp_pool.tile([P, F], F16, name=f"q{t}", tag="q")
                nc.gpsimd.tensor_tensor(out=q, in0=v[4], in1=d, op=OP.mult)

                w = tmp_pool.tile([P, F], F16, name=f"w{t}", tag="w")
                nc.vector.tensor_tensor(out=w, in0=v[0], in1=v[1], op=OP.mult)
                nc.vector.tensor_tensor(
                    out=w, in0=w, in1=stile[:, t * F:(t + 1) * F], op=OP.add)
                nc.vector.tensor_tensor(out=w, in0=w, in1=e, op=OP.mult)
                nc.vector.tensor_tensor(out=w, in0=w, in1=q, op=OP.add)
                nc.vector.tensor_tensor(out=w, in0=w, in1=v[7], op=OP.subtract)
                o = tmp_pool.tile([P, F], F16, name=f"o{t}", tag="o")
                i3 = nc.scalar.activation(o, w, AF.Tanh)

                if last_sin is not None:
                    for bi in (i1, i3):
                        add_dep_helper(bi.ins, last_sin, False,
                                       "act-set phase order")

                nc.sync.dma_start(
                    out=y[r0:r1].rearrange("(p f) -> p f", p=P), in_=o)
    nc.compile()
    return nc


_BUILT = None


def _get_built():
    global _BUILT
    if _BUILT is None:
        _BUILT = build_bass()
    return _BUILT


def make_in_maps(inputs: np.ndarray) -> list[dict]:
    x = np.asarray(inputs, dtype=np.float32)
    assert x.shape == (N_ROWS, N_VARS), x.shape
    xT = np.ascontiguousarray(x.T)           # [8, N]
    xs_all = xT[XS_VARS].astype(np.float16)  # [7, N] var-major
    x2_all = xT[2].astype(np.float16)        # [N]
    R = ROWS_PER_CORE
    return [
        {
            "xs": np.ascontiguousarray(xs_all[:, c * R:(c + 1) * R]),
            "x2": np.ascontiguousarray(x2_all[c * R:(c + 1) * R]),
        }
        for c in range(N_CORES)
    ]


def run_spmd(inputs: np.ndarray, **kwargs) -> tuple[np.ndarray, object]:
    """Shard, run on 8 cores, gather. Retries transient device wedges."""
    import time as _time
    in_maps = make_in_maps(inputs)
    nc = _get_built()
    last_exc = None
    for attempt in range(3):
        try:
            res = bass_utils.run_bass_kernel_spmd(
                nc, in_maps, core_ids=list(range(N_CORES)), **kwargs
            )
            break
        except Exception as exc:  # transient device wedge — retry
            last_exc = exc
            _time.sleep(10 * (attempt + 1))
    else:
        raise last_exc
    out = np.concatenate([r["y"].reshape(-1) for r in res.results], axis=0)
    return out.astype(np.float32), res


def kernel(inputs: np.ndarray) -> np.ndarray:
    out, _ = run_spmd(inputs)
    return out


# revision 11
# speedup vs baseline: 1.1713x; 1.1713x over previous
"""Trainium2 Bass kernel for:
    tanh( (x0*x1 + sin(x2)) * exp(-|x3|) + x4 / (x5*x5 + exp(x6)) - x7 )
over inputs (8388608, 8) f32, data-parallel over 8 NeuronCores.

v3 design (compute-bound after the fp16 shrink; DMA floor ~32 us):
  - Host marshals inputs to var-major fp16 (end-to-end error ~4e-4 vs
    the 2e-2 gate): xs[7, R] holds vars in order [x0,x1,x3,x6,x4,x5,x7]
    so the two exp operands (-|x3| and x6) are ADJACENT -> one fused
    2F-wide ACT exp instead of two F-wide ones. x2[R] fp16 separately.
  - Two ACT table-set phases TOTAL (not per batch): pass A computes
    sin(wrap(x2)) for the whole shard into a resident fp16 buffer
    (silu set), pass B does exp/square/tanh (exp_and_others set).
  - F=2048 tiles (4/core) halve per-instruction overheads.
  - In-place ops throughout (sign-or on the x3 slice, d/recip on the
    sq tile, the 5-op DVE chain on one w tile) keep SBUF under 180 KB
    per partition.
  - Engine balance per tile: ACT exp(2F)+square+tanh; Pool d,q;
    DVE sign-or, recip, 5 fp16 tensor_tensor at 2x mode.
"""

import numpy as np

import concourse.bass as bass
import concourse.bacc as bacc
import concourse.mybir as mybir
from concourse.tile import TileContext
from concourse.tile_rust import add_dep_helper
from concourse import bass_utils

N_ROWS = 8_388_608
N_VARS = 8
N_CORES = 8
ROWS_PER_CORE = N_ROWS // N_CORES  # 1_048_576
P = 128
F = 1024
TILE_ROWS = P * F                  # 131_072
N_TILES = ROWS_PER_CORE // TILE_ROWS  # 8

F32 = mybir.dt.float32
F16 = mybir.dt.float16
U16 = mybir.dt.uint16
AF = mybir.ActivationFunctionType
OP = mybir.AluOpType

# xs row order: x3 and x6 adjacent for the fused exp
XS_VARS = [0, 1, 3, 6, 4, 5, 7]
SLOT = {v: i for i, v in enumerate(XS_VARS)}


def build_bass(loop_iters: int = 1, ablate: str = "none") -> bass.Bass:
    import contextlib
    nc = bacc.Bacc("TRN2", debug=False, num_devices=N_CORES)
    xs = nc.dram_tensor("xs", [7, ROWS_PER_CORE], F16, kind="ExternalInput").ap()
    x2 = nc.dram_tensor("x2", [ROWS_PER_CORE], F16, kind="ExternalInput").ap()
    y = nc.dram_tensor("y", [ROWS_PER_CORE], F16, kind="ExternalOutput").ap()

    with TileContext(nc) as tc:
        with (
            tc.tile_pool(name="sin", bufs=1) as sin_pool,
            tc.tile_pool(name="pa", bufs=3) as pa_pool,
            tc.tile_pool(name="inp", bufs=5) as inp_pool,
            tc.tile_pool(name="tmp", bufs=3) as tmp_pool,
            (tc.For_i(0, loop_iters, 1) if loop_iters > 1
             else contextlib.nullcontext()),
        ):
            stile = sin_pool.tile([P, N_TILES * F], F16, name="stile")

            # ---- Pass A: sin(wrap(x2)) for the whole shard (silu set) ----
            # One up-front 2MB DMA so the sins are never starved behind the
            # big pass-B loads; wrap+sin in 2 wide chunks (fewer ops).
            x2all = sin_pool.tile([P, N_TILES * F], F16, name="x2all")
            nc.sync.dma_start(
                out=x2all.rearrange("p (t f) -> p t f", t=N_TILES),
                in_=x2.rearrange("(t p f) -> p t f", t=N_TILES, p=P))
            last_sin = None
            if ablate != "dma":
                for t in range(N_TILES):
                    sl = slice(t * F, (t + 1) * F)
                    wr = pa_pool.tile([P, F], F16, name=f"wr{t}", tag="wr")
                    nc.vector.add_range_wrap(
                        out=wr, in_=x2all[:, sl], shift=0.0,
                        bound=float(np.pi), period=float(2 * np.pi))
                    si = nc.scalar.activation(stile[:, sl], wr, AF.Sin)
                    last_sin = si.ins

            # ---- Pass B: everything else (exp_and_others set) ----
            for t in range(N_TILES):
                r0, r1 = t * TILE_ROWS, (t + 1) * TILE_ROWS
                xt = inp_pool.tile([P, 7 * F], F16, name=f"xt{t}", tag="xt")
                nc.sync.dma_start(
                    out=xt.rearrange("p (v f) -> p v f", v=7),
                    in_=xs[:, r0:r1].rearrange("v (p f) -> p v f", p=P))
                v = {k: xt[:, s * F:(s + 1) * F] for k, s in SLOT.items()}
                if ablate == "dma":
                    nc.sync.dma_start(
                        out=y[r0:r1].rearrange("(p f) -> p f", p=P),
                        in_=v[7])
                    continue

                # -|x3| in place via sign-bit OR (DVE, 16-bit)
                nc.vector.tensor_scalar(
                    out=v[3].bitcast(U16), in0=v[3].bitcast(U16),
                    scalar1=0x8000, scalar2=None, op0=OP.bitwise_or)

                # fused exp over the adjacent [x3'|x6] 2F slice
                e2 = tmp_pool.tile([P, 2 * F], F16, name=f"e2{t}", tag="e2")
                i1 = nc.scalar.activation(e2, xt[:, 2 * F:4 * F], AF.Exp)
                e, e6 = e2[:, 0:F], e2[:, F:2 * F]

                sq = tmp_pool.tile([P, F], F16, name=f"sq{t}", tag="sq")
                nc.vector.tensor_tensor(out=sq, in0=v[5], in1=v[5], op=OP.mult)

                d = tmp_pool.tile([P, F], F32, name=f"d{t}", tag="d")
                nc.gpsimd.tensor_tensor(out=d, in0=sq, in1=e6, op=OP.add)
                nc.vector.reciprocal_approx_fast(out=d, in_=d)
                q = tmp_pool.tile([P, F], F16, name=f"q{t}", tag="q")
                nc.gpsimd.tensor_tensor(out=q, in0=v[4], in1=d, op=OP.mult)

                w = tmp_pool.tile([P, F], F16, name=f"w{t}", tag="w")
                nc.vector.tensor_tensor(out=w, in0=v[0], in1=v[1], op=OP.mult)
                nc.vector.tensor_tensor(
                    out=w, in0=w, in1=stile[:, t * F:(t + 1) * F], op=OP.add)
                nc.vector.tensor_tensor(out=w, in0=w, in1=e, op=OP.mult)
                nc.vector.tensor_tensor(out=w, in0=w, in1=q, op=OP.add)
                nc.vector.tensor_tensor(out=w, in0=w, in1=v[7], op=OP.subtract)
                o = tmp_pool.tile([P, F], F16, name=f"o{t}", tag="o")
                i3 = nc.scalar.activation(o, w, AF.Tanh)

                if last_sin is not None:
                    for bi in (i1, i3):
                        add_dep_helper(bi.ins, last_sin, False,
                                       "act-set phase order")

                nc.sync.dma_start(
                    out=y[r0:r1].rearrange("(p f) -> p f", p=P), in_=o)
    nc.compile()
    return nc


_BUILT = None


def _get_built():
    global _BUILT
    if _BUILT is None:
        _BUILT = build_bass()
    return _BUILT


def make_in_maps(inputs: np.ndarray) -> list[dict]:
    x = np.asarray(inputs, dtype=np.float32)
    assert x.shape == (N_ROWS, N_VARS), x.shape
    xT = np.ascontiguousarray(x.T)           # [8, N]
    xs_all = xT[XS_VARS].astype(np.float16)  # [7, N] var-major
    x2_all = xT[2].astype(np.float16)        # [N]
    R = ROWS_PER_CORE
    return [
        {
            "xs": np.ascontiguousarray(xs_all[:, c * R:(c + 1) * R]),
            "x2": np.ascontiguousarray(x2_all[c * R:(c + 1) * R]),
        }
        for c in range(N_CORES)
    ]


def run_spmd(inputs: np.ndarray, **kwargs) -> tuple[np.ndarray, object]:
    """Shard, run on 8 cores, gather. Retries transient device wedges."""
    import time as _time
    in_maps = make_in_maps(inputs)
    nc = _get_built()
    last_exc = None
    for attempt in range(3):
        try:
            res = bass_utils.run_bass_kernel_spmd(
                nc, in_maps, core_ids=list(range(N_CORES)), **kwargs
            )
            break
        except Exception as exc:  # transient device wedge — retry
            last_exc = exc
            _time.sleep(10 * (attempt + 1))
    else:
        raise last_exc
    out = np.concatenate([r["y"].reshape(-1) for r in res.results], axis=0)
    return out.astype(np.float32), res


def kernel(inputs: np.ndarray) -> np.ndarray:
    out, _ = run_spmd(inputs)
    return out
